# revision 11
# baseline (speedup 1.0000x reference)
"""Causal self-attention, sharded over 8 NeuronCores (batch x head-group).

B=2, S=2048, D=1024, NH=16, HD=64.  Core c handles batch c//4 and heads
4*(c%4) .. 4*(c%4)+3.  QKV is column-parallel, attention is fully local per
head, the output projection is row-parallel; the 4 partial projections per
batch are summed on the host (along with b_proj).

Device kernel layout notes:
  - x is loaded naturally and transposed on the tensor engine (fp32 DMA
    transpose is unsupported) into xT [c_in=128, c_out=8, S].
  - qT/kT are stored [128, 2, S]: head pair j on partition halves (head 2j
    on partitions 0:64, head 2j+1 on 64:128) so K=64 matmuls have matching
    base partitions.
  - Scores are computed transposed, scoresT[ki, qi], so softmax runs along
    the free dim after exp; the denominator comes for free as row 64 of the
    AV matmul via an appended ones-column on V.
  - Causal mask: only the diagonal 128x128 block of each ki-row needs
    masking (applied multiplicatively post-exp via gpsimd.affine_select).
  - All matmul inputs are bitcast to float32r (full-rate fp32 mode).
"""

import sys

if "/opt/trn_rl_repo" not in sys.path:
    sys.path.insert(0, "/opt/trn_rl_repo")

import ml_dtypes
import numpy as np

import concourse.bass as bass
import concourse.mybir as mybir
import concourse.tile as tile

F32 = mybir.dt.float32
F32R = mybir.dt.float32r
BF16 = mybir.dt.bfloat16
AF = mybir.ActivationFunctionType

S = 2048  # sequence length
D = 1024  # model dim
HC = 4  # heads per core
HD = 64  # head dim
CT = D // 128  # contraction tiles (8)
ST = S // 128  # sequence tiles of 128 (16)
SB = S // 512  # sequence blocks of 512 (4)
SCALE = 0.125  # 1/sqrt(HD)

LAST_RESULT = None  # BassKernelResults of the most recent run (for test.py)





def _split_waits(nc):
    """The walrus build in this container accepts at most ONE sync wait per
    instruction (and none on Matmult). Hoist excess waits onto single-wait
    NoOps inserted just before the instruction on the same engine queue —
    semantically identical (engine stalls on each condition in turn)."""
    n_nops = 0
    for f in nc.m.functions:
        for blk in f.blocks:
            new_insts = []
            for ins in blk.instructions:
                si = getattr(ins, "sync_info", None)
                if si is not None and si.on_wait:
                    keep = 0 if isinstance(ins, mybir.InstMatmult) else 1
                    waits = list(si.on_wait)
                    if len(waits) > keep:
                        excess, si.on_wait = waits[keep:], waits[:keep]
                        for i, w in enumerate(excess):
                            nop = mybir.InstNoOp(
                                name=f"{ins.name}-wn{i}",
                                engine=ins.engine,
                                ins=[],
                                outs=[],
                                sync_info=mybir.SyncInfo(on_wait=[w], on_update=[]),
                            )
                            nc.inst_map[nop.name] = nop
                            new_insts.append(nop)
                            n_nops += 1
                new_insts.append(ins)
            blk.instructions = new_insts
    return n_nops


def _copy(nc, idx, out, in_):
    """PSUM->SBUF copy, alternating DVE and ScalarE (GpSimd cannot read PSUM)."""
    if idx % 2 == 0:
        nc.vector.tensor_copy(out, in_)
    else:
        nc.scalar.copy(out, in_)


def build_program(repeat=1):
    nc = bass.Bass(trn_type="TRN2")

    x = nc.dram_tensor("x", [S, D], F32, kind="ExternalInput")
    wq = nc.dram_tensor("wq", [D, HC * HD], F32R, kind="ExternalInput")
    wk = nc.dram_tensor("wk", [D, HC * HD], F32R, kind="ExternalInput")
    wv = nc.dram_tensor("wv", [D, HC * HD], F32R, kind="ExternalInput")
    wo = nc.dram_tensor("wo", [HC * HD, D], F32R, kind="ExternalInput")
    bqt = nc.dram_tensor("bqt", [128, 2], F32, kind="ExternalInput")
    bkt = nc.dram_tensor("bkt", [128, 2], F32, kind="ExternalInput")
    ident = nc.dram_tensor("ident", [128, 128], F32, kind="ExternalInput")
    onesd = nc.dram_tensor("onesd", [128, 512], BF16, kind="ExternalInput")
    out = nc.dram_tensor("out", [S, D], F32, kind="ExternalOutput")

    with tile.TileContext(nc) as tc:
      for _rep in range(repeat):
        with tc.tile_pool(name="persist", bufs=1) as pers:
            qT = pers.tile([128, 2, S], BF16, tag="qT")
            kT = pers.tile([128, 2, S], BF16, tag="kT")
            vA = pers.tile([128, ST, HC, HD + 1], BF16, tag="vA")
            oT = pers.tile([128, 2, S], F32R, tag="oT")
            bqt_sb = pers.tile([128, 2], F32, tag="bqt")
            bkt_sb = pers.tile([128, 2], F32, tag="bkt")

            nc.sync.dma_start(
                vA[:, :, :, HD : HD + 1],
                onesd[:, 0:64].rearrange("p (s h) -> p s h", s=ST)[:, :, :, None],
            )
            nc.sync.dma_start(bqt_sb[:], bqt[:])
            nc.sync.dma_start(bkt_sb[:], bkt[:])

            # ---------------- phase A: x transpose + QKV ----------------
            with (
                tc.tile_pool(name="phA", bufs=1) as pa,
                tc.tile_pool(name="phA2", bufs=2) as pa2,
                tc.tile_pool(name="psA", bufs=2, space="PSUM") as psA,
            ):
                xT = pa.tile([128, CT, S], F32R, tag="xT")
                idn = pa.tile([128, 128], F32, tag="idn")
                wq_sb = pa.tile([128, CT, HC * HD], F32R, tag="wq")
                wk_sb = pa.tile([128, CT, HC * HD], F32R, tag="wk")
                wv_sb = pa.tile([128, CT, HC * HD], F32R, tag="wv")

                nc.sync.dma_start(idn[:], ident[:])
                for w_dram, w_sb in ((wq, wq_sb), (wk, wk_sb), (wv, wv_sb)):
                    nc.sync.dma_start(
                        w_sb[:], w_dram[:].rearrange("(co ci) f -> ci co f", ci=128)
                    )

                for st in range(ST):
                    xs = pa2.tile([128, D], F32, tag="xs")
                    nc.sync.dma_start(xs[:], x[st * 128 : (st + 1) * 128, :])
                    for cg in range(2):
                        tr = psA.tile([128, 512], F32, tag="tr")
                        for i in range(4):
                            co = cg * 4 + i
                            nc.tensor.transpose(
                                tr[:, i * 128 : (i + 1) * 128],
                                xs[:, co * 128 : (co + 1) * 128],
                                idn[:],
                            )
                        _copy(nc, 2 * st + cg,
                            xT[:, cg * 4 : (cg + 1) * 4, st * 128 : (st + 1) * 128],
                            tr[:].rearrange("p (c s) -> p c s", c=4),
                        )

                # q/k projections: head pair j packed on psum partitions
                for w_sb, b_sb, dstT in ((wq_sb, bqt_sb, qT), (wk_sb, bkt_sb, kT)):
                    for j in range(2):
                        for b in range(SB):
                            ps = psA.tile([128, 512], F32, tag="qk")
                            for ct in range(CT):
                                nc.tensor.matmul(
                                    ps[:],
                                    (w_sb[:, ct, j * 128 : (j + 1) * 128]),
                                    (xT[:, ct, b * 512 : (b + 1) * 512]),
                                    start=(ct == 0),
                                    stop=(ct == CT - 1),
                                )
                            nc.vector.tensor_scalar_add(
                                dstT[:, j, b * 512 : (b + 1) * 512],
                                ps[:],
                                b_sb[:, j : j + 1],
                            )

                # v projection (natural layout), bias folded in via K=1 matmul
                for st in range(ST):
                    ps = psA.tile([128, HC * HD], F32, tag="vps")
                    for ct in range(CT):
                        nc.tensor.matmul(
                            ps[:],
                            (xT[:, ct, st * 128 : (st + 1) * 128]),
                            (wv_sb[:, ct, :]),
                            start=(ct == 0),
                            stop=(ct == CT - 1),
                        )
                    _copy(
                        nc,
                        st,
                        vA[:, st, :, 0:HD],
                        ps[:].rearrange("p (h d) -> p h d", h=HC),
                    )

            # ---------------- phases B+C: attention, then projection ----------------
            with (
                tc.tile_pool(name="phC", bufs=1) as pc,
                tc.tile_pool(name="phC3", bufs=3) as pc3,
            ):
                # wo load early so the DMA overlaps attention
                wo_sb = pc.tile([128, 2, D], F32R, tag="wo")
                nc.sync.dma_start(
                    wo_sb[:], wo[:].rearrange("(co ci) f -> ci co f", ci=128)
                )

                with (
                    tc.tile_pool(name="phB", bufs=2) as pb,
                    tc.tile_pool(name="phB1", bufs=1) as pb1,
                    tc.tile_pool(name="psB", bufs=2, space="PSUM") as psB,
                    tc.tile_pool(name="psAV", bufs=1, space="PSUM") as psAV,
                    tc.tile_pool(name="dramB", bufs=2, space="DRAM") as dramp,
                ):
                    self_attn(nc, tc, qT, kT, vA, oT, pb, pb1, psB, psAV, dramp)

                with tc.tile_pool(name="psC", bufs=2, space="PSUM") as psC:
                    # output projection (row-parallel partial)
                    for st in range(ST):
                        for nb in range(2):
                            ps = psC.tile([128, 512], F32, tag="pj")
                            for kt in range(2):
                                nc.tensor.matmul(
                                    ps[:],
                                    (oT[:, kt, st * 128 : (st + 1) * 128]),
                                    (wo_sb[:, kt, nb * 512 : (nb + 1) * 512]),
                                    start=(kt == 0),
                                    stop=(kt == 1),
                                )
                            ob = pc3.tile([128, 512], F32, tag="ob")
                            _copy(nc, st + nb, ob[:], ps[:])
                            nc.sync.dma_start(
                                out[
                                    st * 128 : (st + 1) * 128,
                                    nb * 512 : (nb + 1) * 512,
                                ],
                                ob[:],
                            )

    _split_waits(nc)
    return nc


def self_attn(nc, tc, qT, kT, vA, oT, pb, pb1, psB, psAV, dramp):
    for h in range(HC):
        j, half = h // 2, h % 2
        base = half * 64
        av = psAV.tile([HD + 1, S], F32, tag="av")
        for t in range(ST):
            span = S - t * 128
            pt = pb.tile([128, S], BF16, tag="probs")
            for c0 in range(0, span, 1024):
                w = min(1024, span - c0)
                sc = psB.tile([128, 1024], F32, tag="sc")
                for n0 in range(0, w, 512):
                    nw = min(512, w - n0)
                    q0 = t * 128 + c0 + n0
                    nc.tensor.matmul(
                        sc[:, n0 : n0 + nw],
                        (kT[base : base + 64, j, t * 128 : (t + 1) * 128]),
                        (qT[base : base + 64, j, q0 : q0 + nw]),
                        start=True,
                        stop=True,
                    )
                nc.scalar.activation(
                    pt[:, c0 : c0 + w], sc[:, :w], AF.Exp, scale=SCALE
                )
            # causal mask on the diagonal 128x128 block: keep qi>=ki
            nc.gpsimd.affine_select(
                out=pt[:, 0:128],
                in_=pt[:, 0:128],
                pattern=[[1, 128]],
                channel_multiplier=-1,
                base=0,
                compare_op=mybir.AluOpType.is_ge,
                fill=0.0,
            )
            for b in range(t // 4, SB):
                lo = max(512 * b, 128 * t)
                hi = 512 * (b + 1)
                nc.tensor.matmul(
                    av[:, lo:hi],
                    (vA[:, t, h, :]),
                    (pt[:, lo - 128 * t : hi - 128 * t]),
                    start=(t == 0),
                    stop=(t == 4 * b + 3),
                    skip_group_check=True,
                )
        # normalize: row HD of av is the softmax denominator
        rc = pb1.tile([HD + 1, S], F32, tag="rc")
        nc.vector.reciprocal(rc[HD : HD + 1, :], av[HD : HD + 1, :])
        rpd = dramp.tile([1, S], F32, tag="rpd")
        nc.sync.dma_start(rpd[:], rc[HD : HD + 1, :])
        rep = pb.tile([64, S], F32, tag="rep")
        nc.sync.dma_start(rep[:], rpd[:].to_broadcast((64, S)))
        if half == 0:
            nc.vector.tensor_mul(oT[0:64, j, :], av[0:64, :], rep[:])
        else:
            hs = pb.tile([64, S], F32R, tag="hs")
            nc.vector.tensor_mul(hs[:], av[0:64, :], rep[:])
            nc.sync.dma_start(oT[64:128, j, :], hs[:])


def make_in_maps(x, w_qkv, b_qkv, w_proj):
    """Per-core input dicts. Core c: batch c//4, heads 4*(c%4)..4*(c%4)+3."""
    x = np.asarray(x, dtype=np.float32)
    w_qkv = np.asarray(w_qkv, dtype=np.float32)
    b_qkv = np.asarray(b_qkv, dtype=np.float32)
    w_proj = np.asarray(w_proj, dtype=np.float32)
    ident = np.eye(128, dtype=np.float32)
    in_maps = []
    for c in range(8):
        b, g = c // 4, c % 4
        cols = slice(g * HC * HD, (g + 1) * HC * HD)
        in_maps.append(
            {
                "x": np.ascontiguousarray(x[b]),
                "wq": np.ascontiguousarray(w_qkv[:, 0:D][:, cols]),
                "wk": np.ascontiguousarray(w_qkv[:, D : 2 * D][:, cols]),
                "wv": np.ascontiguousarray(w_qkv[:, 2 * D : 3 * D][:, cols]),
                "wo": np.ascontiguousarray(w_proj[g * HC * HD : (g + 1) * HC * HD, :]),
                "bqt": np.ascontiguousarray(
                    b_qkv[0:D][cols].reshape(2, 128).T
                ),
                "bkt": np.ascontiguousarray(
                    b_qkv[D : 2 * D][cols].reshape(2, 128).T
                ),
                "ident": ident,
                "onesd": np.ones((128, 512), dtype=ml_dtypes.bfloat16),
            }
        )
    return in_maps


_PROGRAM = None


def kernel(x, w_qkv, b_qkv, w_proj, b_proj):
    global _PROGRAM, LAST_RESULT
    from concourse.bass_utils import run_bass_kernel_spmd

    if _PROGRAM is None:
        _PROGRAM = build_program()
    in_maps = make_in_maps(x, w_qkv, b_qkv, w_proj)
    res = run_bass_kernel_spmd(_PROGRAM, in_maps, core_ids=list(range(8)))
    LAST_RESULT = res
    b_proj = np.asarray(b_proj, dtype=np.float32)
    w_proj = np.asarray(w_proj, dtype=np.float32)
    b_qkv = np.asarray(b_qkv, dtype=np.float32)
    # v-bias commutes through softmax (rows sum to 1): contributes b_v @ w_proj
    const = b_proj + b_qkv[2 * D : 3 * D] @ w_proj
    parts = [r["out"] for r in res.results]
    out = np.stack(
        [
            parts[0] + parts[1] + parts[2] + parts[3] + const,
            parts[4] + parts[5] + parts[6] + parts[7] + const,
        ]
    )
    return out.astype(np.float32)

